# revision 10
# baseline (speedup 1.0000x reference)
"""Causal MHA on 8 trn2 NeuronCores.

Sharding: core c -> batch b=c//4, head group g=c%4 (4 heads = 256 proj cols).
Host preps per-core transposed bf16 inputs; device computes the o_proj
partial product for its head group; host sums the 4 partials per batch.

Device pipeline per core (4 heads, d_k = 64, S = 2048, D = 1024), all
matmuls bf16 with fp32 PSUM accumulation:
  qT/kT = Wq_g @ x_b   as [256, S] bf16 (contraction over D)
  v     = x_b @ Wv_g.T as [S, 256] bf16, augmented with a ones column
  per head, per sq-chunk (512), per sk-tile (128):
    scoresT[sk, sq] = kT_h_tile.T @ qT_h   (K=64, causal rhs slicing)
    attnT = exp(0.125 * scoresT) -> bf16 (no max subtraction; scores ~ N(0,1))
    diagonal tiles: multiply first 128 cols by upper-tri mask
    outT[65, sq] += v'_tile.T @ attnT      (K=128; row 64 = softmax sums)
  normalize: recip(sums) fp32 -> K=1 ones matmul broadcast -> DVE multiply
  o_proj partial[s, n] = outT_all_heads.T @ owT_g, fp32 out, DMA to DRAM
"""

import os

import ml_dtypes
import numpy as np

import concourse.bass as bass
import concourse.mybir as mybir
import concourse.tile as tile
from concourse.bass_utils import run_bass_kernel_spmd

F32 = mybir.dt.float32
BF16 = mybir.dt.bfloat16

B, S, D, H, DK = 2, 2048, 1024, 16, 64
HC = 4          # heads per core
M = HC * DK     # 256 proj columns per core
NK = D // 128   # 8 contraction tiles for projections
NST = S // 128  # 16 sequence tiles
NSC = S // 512  # 4 sequence chunks


def _emit(ctx, tc, io):
    nc = tc.nc
    Exp = mybir.ActivationFunctionType.Exp

    wpool = ctx.enter_context(tc.tile_pool(name="wpool", bufs=1))
    big = ctx.enter_context(tc.tile_pool(name="big", bufs=1))
    at = ctx.enter_context(tc.tile_pool(name="at", bufs=4))
    sm = ctx.enter_context(tc.tile_pool(name="sm", bufs=3))
    obuf = ctx.enter_context(tc.tile_pool(name="obuf", bufs=4))
    ps_p = ctx.enter_context(tc.tile_pool(name="ps_p", bufs=2, space="PSUM"))
    ps_a = ctx.enter_context(tc.tile_pool(name="ps_a", bufs=2, space="PSUM"))
    ps_b = ctx.enter_context(tc.tile_pool(name="ps_b", bufs=3, space="PSUM"))
    ps_r = ctx.enter_context(tc.tile_pool(name="ps_r", bufs=1, space="PSUM"))

    # ---- load inputs (all bf16) ----
    xt_sb = []
    for k in range(NK):
        t = wpool.tile([128, S], BF16, name=f"xt{k}", tag=f"xt{k}")
        nc.sync.dma_start(out=t, in_=io["xt"][128 * k : 128 * (k + 1), :])
        xt_sb.append(t)

    w_sb = {}
    for wname in ("wqt", "wkt", "wvt"):
        tiles = []
        for k in range(NK):
            t = wpool.tile([128, M], BF16, name=f"{wname}{k}", tag=f"{wname}{k}")
            nc.sync.dma_start(out=t, in_=io[wname][128 * k : 128 * (k + 1), :])
            tiles.append(t)
        w_sb[wname] = tiles

    owt_sb = []
    for k in range(2):
        t = wpool.tile([128, D], BF16, name=f"owt{k}", tag=f"owt{k}")
        nc.sync.dma_start(out=t, in_=io["owt"][128 * k : 128 * (k + 1), :])
        owt_sb.append(t)

    tm_sb = wpool.tile([128, 128], BF16, name="tm", tag="tm")
    nc.sync.dma_start(out=tm_sb, in_=io["trimask"])

    ones_sb = wpool.tile([128, DK], F32, name="ones", tag="ones")
    nc.vector.memset(ones_sb, 1.0)

    # ---- q/k projections: qT/kT [256, S] bf16 (head-major rows, 2 tiles) ----
    qt_sb = [big.tile([128, S], BF16, name=f"qt{m}", tag=f"qt{m}") for m in range(2)]
    kt_sb = [big.tile([128, S], BF16, name=f"kt{m}", tag=f"kt{m}") for m in range(2)]
    for wname, dest in (("wqt", qt_sb), ("wkt", kt_sb)):
        for mt in range(2):
            for sc in range(NSC):
                ps = ps_p.tile([128, 512], F32, name="psqk", tag="ps_p")
                for k in range(NK):
                    nc.tensor.matmul(
                        ps,
                        lhsT=w_sb[wname][k][:, 128 * mt : 128 * (mt + 1)],
                        rhs=xt_sb[k][:, 512 * sc : 512 * (sc + 1)],
                        start=(k == 0),
                        stop=(k == NK - 1),
                    )
                nc.vector.tensor_copy(
                    dest[mt][:, 512 * sc : 512 * (sc + 1)], ps
                )

    # ---- v projection: v' [S, 4, 65] bf16 (col 64 = ones) ----
    vp = []
    for st in range(NST):
        t = big.tile([128, HC, DK + 1], BF16, name=f"vp{st}", tag=f"vp{st}")
        ps = ps_p.tile([128, M], F32, name="psv", tag="ps_p")
        for k in range(NK):
            nc.tensor.matmul(
                ps,
                lhsT=xt_sb[k][:, 128 * st : 128 * (st + 1)],
                rhs=w_sb["wvt"][k],
                start=(k == 0),
                stop=(k == NK - 1),
            )
        nc.vector.tensor_copy(
            t[:, :, 0:DK], ps.rearrange("p (h d) -> p h d", h=HC)
        )
        nc.vector.memset(t[:, :, DK : DK + 1], 1.0)
        vp.append(t)

    # ---- attention + o_proj, per sq-chunk ----
    outT = [big.tile([128, S], BF16, name=f"ot{m}", tag=f"ot{m}") for m in range(2)]

    for c in range(NSC):
        for hp in range(2):  # head pair = row tile of qT/kT
            psb = [
                ps_b.tile([128, 512], F32, name=f"psb{ho}", tag="psb")
                for ho in range(2)
            ]
            for u in range(4 * c + 4):  # sk tiles (causal: u <= 4c+3)
                j = u - 4 * c
                sqlo = 128 * j if j >= 0 else 0
                n = 512 - sqlo
                for ho in range(2):
                    h = 2 * hp + ho
                    p0 = 64 * ho
                    psa = ps_a.tile([128, 512], F32, name="psa", tag="ps_a")
                    nc.tensor.matmul(
                        psa[:, :n],
                        lhsT=kt_sb[hp][p0 : p0 + 64, 128 * u : 128 * (u + 1)],
                        rhs=qt_sb[hp][p0 : p0 + 64, 512 * c + sqlo : 512 * (c + 1)],
                        start=True,
                        stop=True,
                    )
                    atile = at.tile([128, 512], BF16, name="atile", tag="at")
                    nc.scalar.activation(atile[:, :n], psa[:, :n], Exp, scale=0.125)
                    if j >= 0:
                        nc.vector.tensor_mul(
                            atile[:, 0:128], atile[:, 0:128], tm_sb
                        )
                    nc.tensor.matmul(
                        psb[ho][0:65, sqlo:512],
                        lhsT=vp[u][:, h, :],
                        rhs=atile[:, :n],
                        start=(u == 0),
                        stop=(u == 4 * c + 3),
                    )
            # normalize: rows 0..63 of psb divided by row 64 (softmax sums)
            for ho in range(2):
                rec = sm.tile([128, 512], F32, name="rec", tag="rec")
                nc.vector.reciprocal(rec[64:65, :], psb[ho][64:65, :])
                psr = ps_r.tile([128, 512], F32, name="psr", tag="ps_r")
                nc.tensor.matmul(
                    psr[0:64, :],
                    lhsT=ones_sb[64:65, 0:DK],
                    rhs=rec[64:65, :],
                    start=True,
                    stop=True,
                )
                recb = sm.tile([64, 512], F32, name="recb", tag="recb")
                nc.vector.tensor_copy(recb, psr[0:64, :])
                if ho == 0:
                    nc.vector.tensor_mul(
                        outT[hp][0:64, 512 * c : 512 * (c + 1)],
                        psb[0][0:64, :],
                        recb,
                    )
                else:
                    tmp = sm.tile([64, 512], BF16, name="tmpo", tag="tmpo")
                    nc.vector.tensor_mul(tmp, psb[1][0:64, :], recb)
                    nc.sync.dma_start(
                        out=outT[hp][64:128, 512 * c : 512 * (c + 1)], in_=tmp
                    )
        # o_proj for the s-tiles of this chunk
        for st in range(4 * c, 4 * c + 4):
            for nck in range(2):
                ps = ps_p.tile([128, 512], F32, name="pso", tag="ps_p")
                for kt in range(2):
                    nc.tensor.matmul(
                        ps,
                        lhsT=outT[kt][:, 128 * st : 128 * (st + 1)],
                        rhs=owt_sb[kt][:, 512 * nck : 512 * (nck + 1)],
                        start=(kt == 0),
                        stop=(kt == 1),
                    )
                ob = obuf.tile([128, 512], F32, name="ob", tag="ob")
                nc.vector.tensor_copy(ob, ps)
                nc.sync.dma_start(
                    out=io["out_p"][
                        128 * st : 128 * (st + 1), 512 * nck : 512 * (nck + 1)
                    ],
                    in_=ob,
                )


def _legalize_single_wait(nc):
    """The cayman TPB instruction struct has one embedded wait slot, and this
    walrus build refuses instructions with more. Hoist extra waits onto
    injected same-engine NoOps directly before each instruction — engine
    queues are strict FIFO, so semantics are preserved."""
    f = nc.m.functions[0]
    for blk in f.blocks:
        insts = blk.instructions  # live list
        i = 0
        while i < len(insts):
            ins = insts[i]
            si = ins.sync_info
            if si is not None and si.on_wait and len(si.on_wait) > 1:
                waits = list(si.on_wait)
                for w in waits[:-1]:
                    nop = mybir.InstNoOp(
                        name=nc.get_next_instruction_name(),
                        engine=ins.engine,
                        bass_nofuse=True,
                        sync_info=mybir.SyncInfo(on_wait=[w], on_update=[]),
                    )
                    insts.insert(i, nop)
                    i += 1
                ins.sync_info = mybir.SyncInfo(
                    on_wait=[waits[-1]], on_update=list(si.on_update or [])
                )
            i += 1


_CACHE = {}


def _build():
    if "nc" in _CACHE:
        return _CACHE["nc"]
    nc = bass.Bass(
        "TRN2",
        target_bir_lowering=False,
        debug=False,
        enable_asserts=False,
        num_devices=8,
    )
    io = {
        "xt": nc.dram_tensor("xt", (D, S), BF16, kind="ExternalInput").ap(),
        "wqt": nc.dram_tensor("wqt", (D, M), BF16, kind="ExternalInput").ap(),
        "wkt": nc.dram_tensor("wkt", (D, M), BF16, kind="ExternalInput").ap(),
        "wvt": nc.dram_tensor("wvt", (D, M), BF16, kind="ExternalInput").ap(),
        "owt": nc.dram_tensor("owt", (M, D), BF16, kind="ExternalInput").ap(),
        "trimask": nc.dram_tensor(
            "trimask", (128, 128), BF16, kind="ExternalInput"
        ).ap(),
        "out_p": nc.dram_tensor("out_p", (S, D), F32, kind="ExternalOutput").ap(),
    }
    from contextlib import ExitStack

    with tile.TileContext(nc) as tc, ExitStack() as ctx:
        _emit(ctx, tc, io)
    _legalize_single_wait(nc)
    _CACHE["nc"] = nc
    return nc


def make_in_maps(x, qw, kw, vw, ow):
    bf = ml_dtypes.bfloat16
    x = np.asarray(x, dtype=np.float32)
    qw = np.asarray(qw, dtype=np.float32)
    kw = np.asarray(kw, dtype=np.float32)
    vw = np.asarray(vw, dtype=np.float32)
    ow = np.asarray(ow, dtype=np.float32)
    trimask = np.triu(np.ones((128, 128))).astype(bf)
    in_maps = []
    for c in range(8):
        b, g = c // 4, c % 4
        sl = slice(M * g, M * (g + 1))
        in_maps.append(
            {
                "xt": np.ascontiguousarray(x[b].T).astype(bf),
                "wqt": np.ascontiguousarray(qw[sl].T).astype(bf),
                "wkt": np.ascontiguousarray(kw[sl].T).astype(bf),
                "wvt": np.ascontiguousarray(vw[sl].T).astype(bf),
                "owt": np.ascontiguousarray(ow[:, sl].T).astype(bf),
                "trimask": trimask,
            }
        )
    return in_maps


def kernel(x, q_proj_weight, k_proj_weight, v_proj_weight, o_proj_weight):
    nc = _build()
    in_maps = make_in_maps(
        x, q_proj_weight, k_proj_weight, v_proj_weight, o_proj_weight
    )
    trace = bool(os.environ.get("KERNEL_TRACE"))
    if trace:
        try:
            from antenv.axon_hooks import get_axon_ntff_profile_hook  # noqa: F401
        except ImportError:
            trace = False
    res = run_bass_kernel_spmd(
        nc, in_maps, core_ids=list(range(8)), trace=trace
    )
    if trace and res.exec_time_ns is not None:
        print(f"HW exec time: {res.exec_time_ns} ns")
        print(f"mean exec time: {res.mean_exec_time_ns} ns")
    parts = [r["out_p"] for r in res.results]
    out = np.stack(
        [
            parts[0] + parts[1] + parts[2] + parts[3],
            parts[4] + parts[5] + parts[6] + parts[7],
        ],
        axis=0,
    )
    return out
